# revision 4
# baseline (speedup 1.0000x reference)
"""Scalar LSTM (I=H=O=1), B=1024, T=16384, followed by pointwise Linear.

Data-parallel over batch across 8 NeuronCores (128 rows/core, one batch row
per SBUF partition). The sequential-in-T recurrence is evaluated with a
Picard/Jacobi fixed-point iteration on the h->gate feedback, fully parallel
over T within a sweep; the c-recurrence is solved exactly each sweep by the
hardware tensor_tensor_scan (fp32 state, carry chained across chunks).

v2 redesign vs the original Gauss-Seidel fp32 version:
  * pure Jacobi with double-buffered H (sweep k reads H[k%2], writes
    H[(k+1)%2]) — removes the cross-chunk serial dependency inside a sweep,
    so DVE/ACT pipeline freely across chunks and adjacent sweeps overlap.
  * bf16 tiles for X/H/u/gates (DVE tensor ops hit the 2x perf mode);
    scan state and c stay fp32.  Error floor ~7e-3 << 2e-2 gate (simulated
    + verified on HW).
  * sweep 0 (H==0) computes gates directly from fp32 x via ACT's built-in
    affine (no DVE pre-op), overlapped with the chunked DMA-in + downcast.
  * c tiles live in PSUM (frees SBUF, ACT reads PSUM slightly faster).
  * last sweep: tanh(c), h and y in fp32 (halves the bf16 floor), y written
    per-chunk and DMA'd out overlapping the tail of compute.
gate order (i, f, g, o); funcs (sig, sig, tanh, sig); u_g = (wih_g/whh_g)*x
+ h so ACT applies gate = act(whh_g * u + beta_g) with its free affine.
"""

import os
import numpy as np

B, T = 1024, 16384
NCORES = 8
BC = B // NCORES          # 128 batch rows per core = SBUF partitions
C = int(os.environ.get("KERNEL_CHUNK", "2048"))   # time-chunk size
K = int(os.environ.get("KERNEL_SWEEPS", "4"))     # total sweeps incl sweep 0
CPSUM = bool(int(os.environ.get("KERNEL_CPSUM", "1")))
UBUFS = int(os.environ.get("KERNEL_UBUFS", "3"))
LASTF32 = bool(int(os.environ.get("KERNEL_LASTF32", "1")))
NCH = T // C

LAST_RESULTS = None       # test.py introspects this for exec_time_ns


def _build_program(wih, whh, beta, W00, b0):
    import concourse.bacc as bacc
    import concourse.mybir as mybir
    from concourse.tile import TileContext

    F32 = mybir.dt.float32
    BF16 = mybir.dt.bfloat16
    AF = mybir.ActivationFunctionType
    OP = mybir.AluOpType

    funcs = [AF.Sigmoid, AF.Sigmoid, AF.Tanh, AF.Sigmoid]
    wt = [0.0] * 4
    for g in range(4):
        assert abs(whh[g]) > 1e-8 * max(1.0, abs(wih[g])), (
            "degenerate w_hh; u=wt*x+h folding invalid"
        )
        wt[g] = float(wih[g] / whh[g])
    v = [float(whh[g]) for g in range(4)]
    bt = [float(beta[g]) for g in range(4)]

    nc = bacc.Bacc(None, target_bir_lowering=False)
    xin = nc.declare_dram_parameter("x", [BC, T], F32, isOutput=False)
    yout = nc.declare_dram_parameter("y", [BC, T], F32, isOutput=True)

    with TileContext(nc) as tc:
        with (
            tc.tile_pool(name="persist", bufs=1) as pp,
            tc.tile_pool(name="xload", bufs=2) as xp,
            tc.tile_pool(name="work", bufs=UBUFS) as wp,
            tc.tile_pool(name="cpool", bufs=2,
                         space="PSUM" if CPSUM else "SBUF") as cp,
            tc.tile_pool(name="ypool", bufs=2) as yp,
        ):
            X = pp.tile([BC, T], BF16)
            H = [pp.tile([BC, T + 1], BF16, name=f"H{i}") for i in range(2)]
            nc.vector.memset(H[0][:, 0:1], 0.0)
            nc.vector.memset(H[1][:, 0:1], 0.0)
            btile = pp.tile([BC, 4], F32)
            for g in range(4):
                nc.vector.memset(btile[:, g:g + 1], bt[g])

            # ---- sweep 0 (h == 0) overlapped with DMA-in + downcast ----
            # gates straight from fp32 x: gate = act(wih_g * x + beta_g)
            cprev = None
            pend = []           # (chunk, U, c) awaiting tanh+h after scan
            for j in range(NCH):
                s, e = j * C, (j + 1) * C
                xf = xp.tile([BC, C], F32, tag="xf")
                nc.sync.dma_start(out=xf[:, :], in_=xin[:, s:e])
                nc.vector.tensor_scalar(
                    out=X[:, s:e], in0=xf[:, :],
                    scalar1=1.0, scalar2=None, op0=OP.mult)
                U = wp.tile([BC, 4 * C], BF16, tag="U")
                for g in range(4):
                    nc.scalar.activation(
                        out=U[:, g * C:(g + 1) * C], in_=xf[:, :],
                        func=funcs[g], bias=btile[:, g:g + 1],
                        scale=float(wih[g]))
                # z = i*g overwrites i block
                nc.vector.tensor_tensor(
                    out=U[:, 0:C], in0=U[:, 0:C], in1=U[:, 2 * C:3 * C],
                    op=OP.mult)
                c = cp.tile([BC, C], F32, tag="c")
                init = 0.0 if j == 0 else cprev[:, C - 1:C]
                nc.vector.tensor_tensor_scan(
                    out=c[:, :], data0=U[:, C:2 * C], data1=U[:, 0:C],
                    initial=init, op0=OP.mult, op1=OP.add)
                cprev = c
                pend.append((j, U, c))
                # software-pipeline the tanh+h one chunk behind the scan
                if len(pend) > 1:
                    _emit_h(nc, pend.pop(0), H[1], C, AF, OP)
            while pend:
                _emit_h(nc, pend.pop(0), H[1], C, AF, OP)

            # ---- sweeps 1..K-1 ----
            for k in range(1, K):
                last = (k == K - 1)
                Hr, Hw = H[k % 2], H[(k + 1) % 2]
                cprev = None
                pend = []
                for j in range(NCH):
                    s, e = j * C, (j + 1) * C
                    U = wp.tile([BC, 4 * C], BF16, tag="U")
                    for g in range(4):
                        nc.vector.scalar_tensor_tensor(
                            out=U[:, g * C:(g + 1) * C], in0=X[:, s:e],
                            scalar=wt[g], in1=Hr[:, s:e],
                            op0=OP.mult, op1=OP.add)
                    for g in range(4):
                        nc.scalar.activation(
                            out=U[:, g * C:(g + 1) * C],
                            in_=U[:, g * C:(g + 1) * C],
                            func=funcs[g], bias=btile[:, g:g + 1], scale=v[g])
                    nc.vector.tensor_tensor(
                        out=U[:, 0:C], in0=U[:, 0:C], in1=U[:, 2 * C:3 * C],
                        op=OP.mult)
                    c = cp.tile([BC, C], F32, tag="c")
                    init = 0.0 if j == 0 else cprev[:, C - 1:C]
                    nc.vector.tensor_tensor_scan(
                        out=c[:, :], data0=U[:, C:2 * C], data1=U[:, 0:C],
                        initial=init, op0=OP.mult, op1=OP.add)
                    cprev = c
                    pend.append((j, U, c))
                    if len(pend) > 1:
                        if last:
                            _emit_y(nc, pend.pop(0), wp, yp, yout,
                                    BC, C, F32, W00, b0, AF, OP, LASTF32)
                        else:
                            _emit_h(nc, pend.pop(0), Hw, C, AF, OP)
                while pend:
                    if last:
                        _emit_y(nc, pend.pop(0), wp, yp, yout,
                                BC, C, F32, W00, b0, AF, OP, LASTF32)
                    else:
                        _emit_h(nc, pend.pop(0), Hw, C, AF, OP)

    if not nc.is_finalized():
        nc.finalize()
    return nc


def _emit_h(nc, item, Hw, C, AF, OP):
    j, U, c = item
    s = j * C
    # tanh(c) overwrites the dead g block; h = o * tanh(c)
    nc.scalar.activation(out=U[:, 2 * C:3 * C], in_=c[:, :], func=AF.Tanh)
    nc.vector.tensor_tensor(
        out=Hw[:, s + 1:s + C + 1], in0=U[:, 3 * C:4 * C],
        in1=U[:, 2 * C:3 * C], op=OP.mult)


def _emit_y(nc, item, wp, yp, yout, BC, C, F32, W00, b0, AF, OP, lastf32):
    j, U, c = item
    s = j * C
    yt = yp.tile([BC, C], F32, tag="yt")
    if lastf32:
        # tanh lands in yt; then yt = (o * W00) * yt; then += b0 (in place)
        nc.scalar.activation(out=yt[:, :], in_=c[:, :], func=AF.Tanh)
        nc.vector.scalar_tensor_tensor(
            out=yt[:, :], in0=U[:, 3 * C:4 * C], scalar=W00,
            in1=yt[:, :], op0=OP.mult, op1=OP.mult)
        nc.vector.tensor_scalar(
            out=yt[:, :], in0=yt[:, :],
            scalar1=b0, scalar2=None, op0=OP.add)
    else:
        nc.scalar.activation(out=U[:, 2 * C:3 * C], in_=c[:, :], func=AF.Tanh)
        nc.vector.tensor_tensor(
            out=U[:, 3 * C:4 * C], in0=U[:, 3 * C:4 * C],
            in1=U[:, 2 * C:3 * C], op=OP.mult)
        nc.vector.tensor_scalar(
            out=yt[:, :], in0=U[:, 3 * C:4 * C],
            scalar1=W00, scalar2=b0, op0=OP.mult, op1=OP.add)
    nc.sync.dma_start(out=yout[:, s:s + C], in_=yt[:, :])


def kernel(x, w_ih, w_hh, b_ih, b_hh, W, b):
    global LAST_RESULTS
    from concourse.bass_utils import run_bass_kernel_spmd

    x2 = np.ascontiguousarray(np.asarray(x, dtype=np.float32).reshape(B, T))
    wih = np.asarray(w_ih, dtype=np.float64).reshape(4)
    whh = np.asarray(w_hh, dtype=np.float64).reshape(4)
    beta = (np.asarray(b_ih, dtype=np.float64).reshape(4)
            + np.asarray(b_hh, dtype=np.float64).reshape(4))
    W00 = float(np.asarray(W, dtype=np.float64).reshape(1)[0])
    b0 = float(np.asarray(b, dtype=np.float64).reshape(1)[0])

    nc = _build_program(wih, whh, beta, W00, b0)

    in_maps = [{"x": x2[kk * BC:(kk + 1) * BC]} for kk in range(NCORES)]
    trace = bool(int(os.environ.get("KERNEL_TRACE", "0")))
    res = run_bass_kernel_spmd(nc, in_maps, list(range(NCORES)), trace=trace)
    LAST_RESULTS = res
    y = np.concatenate([res.results[kk]["y"] for kk in range(NCORES)], axis=0)
    return y.reshape(B, T, 1).astype(np.float32)
